# revision 6
# baseline (speedup 1.0000x reference)
"""Tensor-parallel MultiHeadAttention kernel for 8 Trainium2 NeuronCores.

Problem (hardcoded): B=2, N=2048, C=1024, H=16 heads, D=64.
Sharding: core c handles batch b = c//4 and head group hg = c%4
(heads 4*hg .. 4*hg+3).  Each core computes Q/K/V for its 4 heads
(feature-major "transposed" layouts), full attention for those heads,
and a partial output projection; the host sums the 4 partials per batch
and adds the output bias.

Device compute: bf16 matmul operands, fp32 PSUM accumulation, fp32
softmax (exp on ScalarE, no max-subtraction: scores are ~N(0,1)).
"""

import numpy as np
import ml_dtypes

B, N, C, H, D = 2, 2048, 1024, 16, 64
NCORES = 8
HPC = 4            # heads per core
DL = HPC * D       # 256 local feature dim
NB = N // 128      # 16 token blocks
QC = N // 512      # 4 query chunks

_cache: dict = {}


def _patch_drain_cap():
    """The walrus build in this container rejects instructions carrying
    more than a couple of sync-wait commands.  Split excess waits onto
    same-engine NoOps emitted just before the offending instruction."""
    import concourse.mybir as mybir
    from concourse.tile import TileContext
    from concourse.vector_clock import ScopedClock

    if getattr(TileContext, "_drain_cap_patched", False):
        return
    CAP = 1

    orig_commit = TileContext._commit_instruction

    def commit_split(self, inst, lazy_reg_writes=True):
        si = getattr(inst, "sync_info", None)
        if si is not None and si.on_wait is not None and len(si.on_wait) > CAP:
            waits = list(si.on_wait)
            keep = waits[len(waits) - CAP:]
            extra = waits[:len(waits) - CAP]
            for i in range(0, len(extra), CAP):
                nop = mybir.InstNoOp(
                    name=self.nc.get_next_instruction_name(),
                    engine=inst.engine,
                    sync_info=mybir.SyncInfo(on_wait=extra[i:i + CAP],
                                             on_update=[]),
                    bass_nofuse=True,
                )
                orig_commit(self, nop, lazy_reg_writes)
            inst.sync_info = mybir.SyncInfo(
                on_wait=keep, on_update=list(si.on_update))
        return orig_commit(self, inst, lazy_reg_writes)

    TileContext._commit_instruction = commit_split

    def patched(self, tick_clock, wait_clock):
        nc = self.nc
        drain_inst = nc.sync.drain()
        wait_clock.add_sem_waits(
            drain_inst.ins, ScopedClock({None: tick_clock.global_clock})
        )
        si = drain_inst.ins.sync_info
        if si is not None and len(si.on_wait) > CAP:
            waits = list(si.on_wait)
            drain_inst.ins.sync_info = mybir.SyncInfo(
                on_wait=waits[:CAP], on_update=list(si.on_update)
            )
            for i in range(CAP, len(waits), CAP):
                nop_bi = nc.sync.nop(nofuse=True)
                nop_bi.ins.sync_info = mybir.SyncInfo(
                    on_wait=waits[i : i + CAP], on_update=[]
                )
        nc.all_engine_barrier()
        assert self.sems is not None
        popped = nc._tile_sem_poison_stack.pop()
        assert popped is self._sem_poison
        nc.clear_and_free_semaphores(list(self.sems.allocated().values()))
        nc.all_engine_barrier()

    TileContext._drain_and_barrier = patched
    TileContext._drain_cap_patched = True


def _build():
    import concourse.bass as bass
    import concourse.mybir as mybir
    from concourse.tile import TileContext
    from contextlib import ExitStack

    _patch_drain_cap()

    f32 = mybir.dt.float32
    f32r = mybir.dt.float32r
    bf16 = mybir.dt.bfloat16
    AF = mybir.ActivationFunctionType

    nc = bass.Bass()
    xt_p = nc.declare_dram_parameter("xt", [C, N], bf16, isOutput=False)
    wq_p = nc.declare_dram_parameter("wqT", [C, DL], bf16, isOutput=False)
    wk_p = nc.declare_dram_parameter("wkT", [C, DL], bf16, isOutput=False)
    wv_p = nc.declare_dram_parameter("wvT", [C, DL], bf16, isOutput=False)
    wo_p = nc.declare_dram_parameter("woT", [DL, C], bf16, isOutput=False)
    bq_p = nc.declare_dram_parameter("bq", [128, 2], f32, isOutput=False)
    bk_p = nc.declare_dram_parameter("bk", [128, 2], f32, isOutput=False)
    bv_p = nc.declare_dram_parameter("bv", [1, DL], bf16, isOutput=False)
    out_p = nc.declare_dram_parameter("out", [N, C], f32, isOutput=True)

    with TileContext(nc) as tc, ExitStack() as ctx:
        # ---- long-lived SBUF pools ----
        wpool = ctx.enter_context(tc.tile_pool(name="w", bufs=1))
        qkpool = ctx.enter_context(tc.tile_pool(name="qk", bufs=1))
        vpool = ctx.enter_context(tc.tile_pool(name="v", bufs=1))
        otpool = ctx.enter_context(tc.tile_pool(name="ot", bufs=1))
        ptpool = ctx.enter_context(tc.tile_pool(name="pt", bufs=3))
        sepool = ctx.enter_context(tc.tile_pool(name="se", bufs=2))
        recpool = ctx.enter_context(tc.tile_pool(name="rec", bufs=2))
        tmppool = ctx.enter_context(tc.tile_pool(name="tmp", bufs=2))
        xpool = ctx.enter_context(tc.tile_pool(name="x", bufs=16))
        obpool = ctx.enter_context(tc.tile_pool(name="ob", bufs=4))

        # ---- weights / constants into SBUF ----
        wq_sb = wpool.tile([128, 8 * DL], bf16, tag="wq")
        wk_sb = wpool.tile([128, 8 * DL], bf16, tag="wk")
        wv_sb = wpool.tile([128, 8 * DL], bf16, tag="wv")
        wo_sb = wpool.tile([128, 2 * C], bf16, tag="wo")
        for cc in range(8):
            nc.sync.dma_start(out=wq_sb[:, cc * DL:(cc + 1) * DL],
                              in_=wq_p[cc * 128:(cc + 1) * 128, :])
            nc.sync.dma_start(out=wk_sb[:, cc * DL:(cc + 1) * DL],
                              in_=wk_p[cc * 128:(cc + 1) * 128, :])
            nc.sync.dma_start(out=wv_sb[:, cc * DL:(cc + 1) * DL],
                              in_=wv_p[cc * 128:(cc + 1) * 128, :])
        for pr in range(2):
            nc.sync.dma_start(out=wo_sb[:, pr * C:(pr + 1) * C],
                              in_=wo_p[pr * 128:(pr + 1) * 128, :])
        bq_sb = wpool.tile([128, 2], f32, tag="bq")
        bk_sb = wpool.tile([128, 2], f32, tag="bk")
        bv_sb = wpool.tile([1, DL], bf16, tag="bv")
        nc.sync.dma_start(out=bq_sb[:], in_=bq_p[:])
        nc.sync.dma_start(out=bk_sb[:], in_=bk_p[:])
        nc.sync.dma_start(out=bv_sb[:], in_=bv_p[:])
        ones_col = wpool.tile([128, 1], bf16, tag="oc")   # sum-matmul lhsT
        ones_row = wpool.tile([1, 128], bf16, tag="or")   # v-bias lhsT
        ones_b64 = wpool.tile([65, 64], f32, tag="o64")   # bcast lhsT (row 64)
        nc.vector.memset(ones_col[:], 1.0)
        nc.vector.memset(ones_row[:], 1.0)
        nc.vector.memset(ones_b64[:], 1.0)

        # feature-major Q^T,K^T per head-pair; token-major V
        QT = [qkpool.tile([128, N], bf16, tag=f"qt{p}", name=f"QT{p}") for p in range(2)]
        KT = [qkpool.tile([128, N], bf16, tag=f"kt{p}", name=f"KT{p}") for p in range(2)]
        V_sb = vpool.tile([128, NB * DL], bf16, tag="v")
        OT = [otpool.tile([128, N], bf16, tag=f"ot{p}", name=f"OT{p}") for p in range(2)]

        # ---- phase 1: QKV projection ----
        with tc.tile_pool(name="psqk", bufs=4, space="PSUM") as psqk, \
             tc.tile_pool(name="psv", bufs=2, space="PSUM") as psv_pool:
            for nch in range(QC):  # 512-token chunks
                xts = []
                for cc in range(8):
                    xt_t = xpool.tile([128, 512], bf16, tag="xt")
                    nc.sync.dma_start(
                        out=xt_t[:],
                        in_=xt_p[cc * 128:(cc + 1) * 128,
                                 nch * 512:(nch + 1) * 512])
                    xts.append(xt_t)
                for pr in range(2):
                    psq = psqk.tile([128, 512], f32, tag="psqk")
                    psk = psqk.tile([128, 512], f32, tag="psqk")
                    for cc in range(8):
                        nc.tensor.matmul(
                            psq[:], wq_sb[:, cc * DL + pr * 128:cc * DL + (pr + 1) * 128],
                            xts[cc][:], start=(cc == 0), stop=(cc == 7))
                    for cc in range(8):
                        nc.tensor.matmul(
                            psk[:], wk_sb[:, cc * DL + pr * 128:cc * DL + (pr + 1) * 128],
                            xts[cc][:], start=(cc == 0), stop=(cc == 7))
                    nc.scalar.activation(
                        QT[pr][:, nch * 512:(nch + 1) * 512], psq[:],
                        AF.Identity, bias=bq_sb[:, pr:pr + 1], scale=0.125)
                    nc.scalar.activation(
                        KT[pr][:, nch * 512:(nch + 1) * 512], psk[:],
                        AF.Identity, bias=bk_sb[:, pr:pr + 1], scale=1.0)
                for nbl in range(4):  # 128-token blocks
                    nb = nch * 4 + nbl
                    psv = psv_pool.tile([128, DL], f32, tag="psv")
                    for cc in range(8):
                        nc.tensor.matmul(
                            psv[:], xts[cc][:, nbl * 128:(nbl + 1) * 128],
                            wv_sb[:, cc * DL:(cc + 1) * DL],
                            start=(cc == 0), stop=False)
                    nc.tensor.matmul(psv[:], ones_row[:], bv_sb[:],
                                     start=False, stop=True)
                    nc.vector.tensor_copy(V_sb[:, nb * DL:(nb + 1) * DL], psv[:])

        # ---- phase 2: attention per head ----
        with tc.tile_pool(name="pss", bufs=1, space="PSUM") as pss, \
             tc.tile_pool(name="pso", bufs=4, space="PSUM") as pso:
            for hl in range(HPC):
                pr, off = hl // 2, 64 * (hl % 2)
                po = [pso.tile([65, 512], f32, tag="o", name=f"po{hl}_{i}") for i in range(QC)]
                for kb in range(NB):
                    ps = pss.tile([128, N], f32, tag="s")
                    for qc in range(QC):
                        nc.tensor.matmul(
                            ps[:, qc * 512:(qc + 1) * 512],
                            KT[pr][off:off + 64, kb * 128:(kb + 1) * 128],
                            QT[pr][off:off + 64, qc * 512:(qc + 1) * 512],
                            start=True, stop=True)
                    pt = ptpool.tile([128, N], bf16, tag="pt")
                    nc.scalar.activation(pt[:], ps[:], AF.Exp)
                    for qc in range(QC):
                        nc.tensor.matmul(
                            po[qc][0:64, :],
                            V_sb[:, kb * DL + hl * 64:kb * DL + (hl + 1) * 64],
                            pt[:, qc * 512:(qc + 1) * 512],
                            start=(kb == 0), stop=(kb == NB - 1))
                        nc.tensor.matmul(
                            po[qc][64:65, :], ones_col[:],
                            pt[:, qc * 512:(qc + 1) * 512],
                            start=(kb == 0), stop=(kb == NB - 1))
                # softmax denominator -> broadcast -> normalize
                se = sepool.tile([65, N], f32, tag="se")
                for qc in range(QC):
                    nc.vector.tensor_copy(se[64:65, qc * 512:(qc + 1) * 512],
                                          po[qc][64:65, :])
                bc = pss.tile([64, N], f32, tag="s")
                for qc in range(QC):
                    nc.tensor.matmul(
                        bc[:, qc * 512:(qc + 1) * 512],
                        ones_b64[64:65, :],
                        se[64:65, qc * 512:(qc + 1) * 512],
                        start=True, stop=True)
                rec = recpool.tile([64, N], f32, tag="rec")
                nc.vector.reciprocal(rec[:], bc[:])
                if hl % 2 == 0:
                    for qc in range(QC):
                        nc.vector.tensor_mul(
                            OT[pr][0:64, qc * 512:(qc + 1) * 512],
                            po[qc][0:64, :], rec[:, qc * 512:(qc + 1) * 512])
                else:
                    tmp = tmppool.tile([64, N], bf16, tag="tmp")
                    for qc in range(QC):
                        nc.vector.tensor_mul(
                            tmp[:, qc * 512:(qc + 1) * 512],
                            po[qc][0:64, :], rec[:, qc * 512:(qc + 1) * 512])
                    nc.sync.dma_start(out=OT[pr][64:128, :], in_=tmp[:])

            # ---- phase 3: output projection (partial) ----
            for nb in range(NB):
                for cc2 in range(2):
                    pc = pso.tile([128, 512], f32, tag="o")
                    nc.tensor.matmul(pc[:], OT[0][:, nb * 128:(nb + 1) * 128],
                                     wo_sb[:, cc2 * 512:(cc2 + 1) * 512],
                                     start=True, stop=False)
                    nc.tensor.matmul(pc[:], OT[1][:, nb * 128:(nb + 1) * 128],
                                     wo_sb[:, C + cc2 * 512:C + (cc2 + 1) * 512],
                                     start=False, stop=True)
                    ob = obpool.tile([128, 512], f32, tag="ob")
                    nc.vector.tensor_copy(ob[:], pc[:])
                    nc.sync.dma_start(
                        out=out_p[nb * 128:(nb + 1) * 128,
                                  cc2 * 512:(cc2 + 1) * 512],
                        in_=ob[:])
    return nc


def _prep_in_maps(x, qkv_w, qkv_b, out_w):
    bf = ml_dtypes.bfloat16
    in_maps = []
    for c in range(NCORES):
        b, hg = c // 4, c % 4
        h0 = 4 * hg
        qsl = slice(h0 * D, (h0 + 4) * D)
        ksl = slice(C + h0 * D, C + (h0 + 4) * D)
        vsl = slice(2 * C + h0 * D, 2 * C + (h0 + 4) * D)
        in_maps.append({
            "xt": np.ascontiguousarray(x[b].T).astype(bf),
            "wqT": np.ascontiguousarray(qkv_w[qsl].T).astype(bf),
            "wkT": np.ascontiguousarray(qkv_w[ksl].T).astype(bf),
            "wvT": np.ascontiguousarray(qkv_w[vsl].T).astype(bf),
            "woT": np.ascontiguousarray(out_w[:, h0 * D:(h0 + 4) * D].T).astype(bf),
            "bq": np.ascontiguousarray(
                (qkv_b[qsl] * 0.125).reshape(2, 128).T).astype(np.float32),
            "bk": np.ascontiguousarray(
                qkv_b[ksl].reshape(2, 128).T).astype(np.float32),
            "bv": qkv_b[vsl].reshape(1, DL).astype(bf),
        })
    return in_maps


def kernel(x, qkv_w, qkv_b, out_w, out_b):
    from concourse.bass_utils import run_bass_kernel_spmd

    x = np.asarray(x, dtype=np.float32)
    qkv_w = np.asarray(qkv_w, dtype=np.float32)
    qkv_b = np.asarray(qkv_b, dtype=np.float32)
    out_w = np.asarray(out_w, dtype=np.float32)
    out_b = np.asarray(out_b, dtype=np.float32)

    if "nc" not in _cache:
        _cache["nc"] = _build()
    in_maps = _prep_in_maps(x, qkv_w, qkv_b, out_w)
    res = run_bass_kernel_spmd(_cache["nc"], in_maps, list(range(NCORES)))
    out = np.zeros((B, N, C), np.float32)
    for c in range(NCORES):
        out[c // 4] += res.results[c]["out"]
    out += out_b[None, None, :]
    return out
